# revision 37
# baseline (speedup 1.0000x reference)
"""FLA gated linear attention (chunked) for Trainium2, 8-core SPMD.

Sharding: 8 cores = B(2) x H(4); each core handles one (batch, head) pair:
  - head-sliced q/k/v/g projections + low-rank gate projection (fused on host
    into one [D, DK] matrix),
  - chunked gated linear attention recurrence (superchunks of 256 positions),
  - fused RMSNorm * swish gate,
  - row-parallel output projection producing a [T, D] partial; host sums the
    4 head-partials per batch.

Self-contained: hardcodes all shapes; host-side work is only sharding/layout
(slices, transpose, folding constant diagonal scales into weight slices).
"""
import sys
sys.path.insert(0, "/opt/trn_rl_repo")

import numpy as np

B, T, D = 2, 2048, 1024
H = 4
DK, DV = 128, 256
SC, NSC = 256, 8          # superchunk size / count
KT = 8                    # 128-row k-tiles over D
TT = 16                   # 128-row t-tiles over T
NORM = 16.0               # gate logit normalizer
EPS = 1e-6

_CACHE = {}


def _build_program():
    import concourse.tile as tile
    from concourse import bacc, hw_specs, mybir
    from concourse.bass import _add_dep_helper

    # Collapse the ACT piecewise-table choice to a single combined
    # Ln+Exp+Square+Copy set (positions preserved so set ids stay valid).
    # Without this, Exp picks `exp_and_others` while Ln picks `natural_log`,
    # and the scheduler inserts a table reload on nearly every Exp<->Ln
    # transition (~50 loads, ~65us of ACT time). All activations used below
    # (Exp, Ln, Square, Copy) live in natural_log_exp_and_others.
    _keep = {"natural_log_exp_and_others"}
    _orig_tables = hw_specs.get_activation_tables("gen3")
    _filtered = {n: (s if n in _keep else set()) for n, s in _orig_tables.items()}
    bacc.get_activation_tables = lambda arch: _filtered

    FR = mybir.dt.float32r
    F32 = mybir.dt.float32
    AL = mybir.AluOpType
    ACT = mybir.ActivationFunctionType

    nc = bacc.Bacc()

    xt_d = nc.dram_tensor("xt", (D, T), FR, kind="ExternalInput")
    wq_d = nc.dram_tensor("wq", (D, DK), FR, kind="ExternalInput")
    wk_d = nc.dram_tensor("wk", (D, DK), FR, kind="ExternalInput")
    wv_d = nc.dram_tensor("wv", (D, DV), FR, kind="ExternalInput")
    wg_d = nc.dram_tensor("wg", (D, DV), FR, kind="ExternalInput")
    wz_d = nc.dram_tensor("wz", (D, DK), FR, kind="ExternalInput")
    bz_d = nc.dram_tensor("bz", (DK, 1), F32, kind="ExternalInput")
    wo_d = nc.dram_tensor("wo", (DV, D), FR, kind="ExternalInput")
    out_d = nc.dram_tensor("out", (T, D), F32, kind="ExternalOutput")

    with tile.TileContext(nc) as tc:
        with (
            tc.tile_pool(name="consts", bufs=1) as consts,
            tc.tile_pool(name="wpool", bufs=1) as wpool,
            tc.tile_pool(name="persist", bufs=1) as pers,
            tc.tile_pool(name="small", bufs=4) as small,
            tc.tile_pool(name="xt", bufs=2) as xtp,
            tc.tile_pool(name="stage", bufs=3) as stage,
            tc.tile_pool(name="ps_proj", bufs=2, space="PSUM") as psp,
            tc.tile_pool(name="ps_tp", bufs=1, space="PSUM") as pstp,
            tc.tile_pool(name="ps_pa", bufs=1, space="PSUM") as pspa,
            tc.tile_pool(name="ps_po", bufs=1, space="PSUM") as pspo,
            tc.tile_pool(name="ps_pd", bufs=1, space="PSUM") as pspd,
            tc.tile_pool(name="ps_out", bufs=2, space="PSUM") as psout,
        ):
            # ---- constants (inline DRAM -> SBUF) ----
            ident_d = nc.inline_tensor(np.eye(128, dtype=np.float32), name="ident_c")
            jj = np.arange(128)[:, None]
            ii = np.arange(SC)[None, :]
            m0_np = (jj <= ii).astype(np.float32)          # j-tile 0
            m1_np = (jj + 128 <= ii).astype(np.float32)    # j-tile 1
            m0_d = nc.inline_tensor(m0_np, name="m0_c")
            m1_d = nc.inline_tensor(m1_np, name="m1_c")
            zeros_d = nc.inline_tensor(np.zeros((128, 3 * DV), np.float32),
                                       name="zeros_c")
            ident = consts.tile([128, 128], FR)
            m0 = consts.tile([128, SC], F32)
            m1 = consts.tile([128, SC], F32)
            nc.sync.dma_start(ident, ident_d[:, :].bitcast(FR))
            nc.sync.dma_start(m0, m0_d[:, :])
            nc.sync.dma_start(m1, m1_d[:, :])
            eps_t = consts.tile([128, 1], F32)
            nc.vector.memset(eps_t, EPS)
            neginf = consts.tile([128, SC], F32)
            nc.vector.memset(neginf, -3.0e38)
            bz_sb = consts.tile([128, 1], F32)
            nc.sync.dma_start(bz_sb, bz_d[:, :])
            s_abc = pers.tile([128, 3 * DV], FR)    # triple-buffered state
            nc.sync.dma_start(s_abc, zeros_d[:, :].bitcast(FR))

            xt3 = xt_d.rearrange("(k p) t -> p k t", p=128)

            # first x quarter split and interleaved with the weight DMAs so
            # the PE can start the first q-projection ~6us in
            xqs = [xtp.tile([128, KT, 512], FR, tag="xq", name=f"xq{i}")
                   for i in range(4)]
            wq_sb = wpool.tile([128, KT, DK], FR)
            wk_sb = wpool.tile([128, KT, DK], FR)
            wz_sb = wpool.tile([128, KT, DK], FR)
            wv_sb = wpool.tile([128, KT, DV], FR)
            wg_sb = wpool.tile([128, KT, DV], FR)
            wo_sb = wpool.tile([128, 2, D], FR)
            wv3 = wv_d.rearrange("(k p) n -> p k n", p=128)
            wg3 = wg_d.rearrange("(k p) n -> p k n", p=128)
            nc.sync.dma_start(xqs[0][:, 0:4, :], xt3[:, 0:4, 0:512])
            nc.sync.dma_start(wq_sb, wq_d.rearrange("(k p) n -> p k n", p=128))
            nc.sync.dma_start(xqs[0][:, 4:8, :], xt3[:, 4:8, 0:512])
            nc.sync.dma_start(wk_sb, wk_d.rearrange("(k p) n -> p k n", p=128))
            nc.sync.dma_start(wz_sb, wz_d.rearrange("(k p) n -> p k n", p=128))
            nc.sync.dma_start(wv_sb[:, 0:4, :], wv3[:, 0:4, :])
            nc.sync.dma_start(wg_sb[:, 0:4, :], wg3[:, 0:4, :])
            nc.sync.dma_start(wv_sb[:, 4:8, :], wv3[:, 4:8, :])
            nc.sync.dma_start(wg_sb[:, 4:8, :], wg3[:, 4:8, :])

            # ---- persistent activations ----
            qg = pers.tile([128, T], FR)   # q^T, then qg^T in place
            kg = pers.tile([128, T], FR)   # k^T, then kg^T in place
            sp = pers.tile([128, T], F32)  # softplus, cumsum, exp(-G) in place
            eg = pers.tile([128, T], F32)  # exp(G)
            v_sb = pers.tile([128, TT, DV], FR)
            sg_sb = pers.tile([128, TT, DV], F32)
            kbar = pers.tile([128, NSC, SC], FR)   # [t within tile, sc, 2*DK]
            og_sb = pers.tile([128, TT, DV], FR)
            ogt = pers.tile([128, 2, T], FR)       # gated output transposed
            spl = pers.tile([128, NSC], F32)
            elast = pers.tile([128, NSC], F32)

            # ====== projections + gate prep, per T-quarter of 512 ======
            for q4 in range(4):
                tsl = slice(q4 * 512, (q4 + 1) * 512)
                xq = xqs[q4]
                if q4 + 1 < 4:
                    nc.sync.dma_start(
                        xqs[q4 + 1], xt3[:, :, (q4 + 1) * 512:(q4 + 2) * 512])
                # transposed projections: q^T, k^T, z^T -> softplus
                pq = psp.tile([128, 512], F32, tag="pp")
                for k in range(KT):
                    nc.tensor.matmul(pq, wq_sb[:, k, :], xq[:, k, :],
                                     start=(k == 0), stop=(k == KT - 1))
                nc.vector.tensor_copy(qg[:, tsl], pq)
                pk = psp.tile([128, 512], F32, tag="pp")
                for k in range(KT):
                    nc.tensor.matmul(pk, wk_sb[:, k, :], xq[:, k, :],
                                     start=(k == 0), stop=(k == KT - 1))
                nc.vector.tensor_copy(kg[:, tsl], pk)
                pz = psp.tile([128, 512], F32, tag="pp")
                for k in range(KT):
                    nc.tensor.matmul(pz, wz_sb[:, k, :], xq[:, k, :],
                                     start=(k == 0), stop=(k == KT - 1))
                # sp = softplus(-(z + b)) = ln(1 + exp(-(z + b)))
                nc.scalar.activation(sp[:, tsl], pz, ACT.Exp,
                                     bias=bz_sb, scale=-1.0)
                nc.scalar.activation(sp[:, tsl], sp[:, tsl], ACT.Ln, bias=1.0)
                # per-superchunk cumsum of softplus (two superchunks/quarter)
                for s in (2 * q4, 2 * q4 + 1):
                    ssl = slice(s * SC, (s + 1) * SC)
                    nc.vector.tensor_tensor_scan(
                        sp[:, ssl], sp[:, ssl], neginf, 0.0, AL.add, AL.max)
                # SP at superchunk ends, decay factors
                sp3 = sp.rearrange("p (s c) -> p s c", c=SC)
                nc.scalar.copy(
                    spl[:, 2 * q4:2 * q4 + 2].rearrange("p (s o) -> p s o", o=1),
                    sp3[:, 2 * q4:2 * q4 + 2, SC - 1:SC])
                nc.scalar.activation(elast[:, 2 * q4:2 * q4 + 2],
                                     spl[:, 2 * q4:2 * q4 + 2],
                                     ACT.Exp, scale=-1.0 / NORM)
                # qg = q^T * exp(G); kg = k^T * exp(-G) (in place)
                nc.scalar.activation(eg[:, tsl], sp[:, tsl], ACT.Exp,
                                     scale=-1.0 / NORM)
                nc.vector.tensor_mul(qg[:, tsl], qg[:, tsl].bitcast(F32),
                                     eg[:, tsl])
                nc.scalar.activation(sp[:, tsl], sp[:, tsl], ACT.Exp,
                                     scale=1.0 / NORM)
                nc.vector.tensor_mul(kg[:, tsl], kg[:, tsl].bitcast(F32),
                                     sp[:, tsl])
                # k_bar^T = kg^T * elast, transposed to [t, dk] via PE identity
                for s in (2 * q4, 2 * q4 + 1):
                    ssl = slice(s * SC, (s + 1) * SC)
                    kbt = small.tile([128, SC], FR, tag="kbt")
                    nc.vector.tensor_scalar_mul(kbt, kg[:, ssl].bitcast(F32),
                                                elast[:, s:s + 1])
                    ptp = pstp.tile([128, SC], F32, tag="tp")
                    for i in range(2):
                        isl = slice(i * 128, (i + 1) * 128)
                        nc.tensor.matmul(ptp[:, isl], kbt[:, isl], ident,
                                         start=True, stop=True)
                    nc.vector.tensor_copy(kbar[:, s, :], ptp)
                # normal-layout projections: v, g (+ silu via exp/ln chain)
                for i in range(4):
                    tt = q4 * 4 + i
                    xsl = slice(i * 128, (i + 1) * 128)
                    pv = psp.tile([128, DV], F32, tag="pp")
                    for k in range(KT):
                        nc.tensor.matmul(pv, xq[:, k, xsl], wv_sb[:, k, :],
                                         start=(k == 0), stop=(k == KT - 1))
                    nc.vector.tensor_copy(v_sb[:, tt, :], pv)
                    pg = psp.tile([128, DV], F32, tag="pp")
                    for k in range(KT):
                        nc.tensor.matmul(pg, xq[:, k, xsl], wg_sb[:, k, :],
                                         start=(k == 0), stop=(k == KT - 1))
                    nc.scalar.copy(sg_sb[:, tt, :], pg)
                    # sigma(g) = exp(-ln(1 + exp(-g))) — stays in the ln/exp
                    # act table; then silu = g * sigma(g) on DVE
                    sgs = small.tile([128, DV], F32, tag="sgs")
                    nc.scalar.activation(sgs, pg, ACT.Exp, scale=-1.0)
                    nc.scalar.activation(sgs, sgs, ACT.Ln, bias=1.0)
                    nc.scalar.activation(sgs, sgs, ACT.Exp, scale=-1.0)
                    nc.vector.tensor_mul(sg_sb[:, tt, :], sgs, sg_sb[:, tt, :])
                if q4 == 0:
                    nc.sync.dma_start(
                        wo_sb, wo_d.rearrange("(k p) n -> p k n", p=128))

            # ====== recurrence + epilogue + output projection, per SC ======
            for s in range(NSC):
                ssl = slice(s * SC, (s + 1) * SC)
                s_cur = s_abc[:, (s % 3) * DV:(s % 3) * DV + DV]
                s_nxt = s_abc[:, ((s + 1) % 3) * DV:((s + 1) % 3) * DV + DV]
                # intra-chunk scores A^T[j, i], masked to j <= i
                pa = pspa.tile([128, 512], F32, tag="pa")
                am = small.tile([128, 2, SC], FR, tag="am")
                for jt in range(2):
                    jsl = slice(s * SC + jt * 128, s * SC + jt * 128 + 128)
                    nc.tensor.matmul(pa[:, jt * SC:jt * SC + SC],
                                     kg[:, jsl], qg[:, ssl],
                                     start=True, stop=True)
                    nc.vector.tensor_mul(am[:, jt, :],
                                         pa[:, jt * SC:jt * SC + SC],
                                         (m0, m1)[jt])
                # o = qg @ S_prev + tril(A) @ v
                po = pspo.tile([128, 512], F32, tag="po")
                for it in range(2):
                    osl = slice(it * DV, it * DV + DV)
                    isl = slice(s * SC + it * 128, s * SC + it * 128 + 128)
                    nc.tensor.matmul(po[:, osl], qg[:, isl], s_cur,
                                     start=True, stop=False)
                    for jt in range(it + 1):
                        nc.tensor.matmul(
                            po[:, osl],
                            am[:, jt, it * 128:it * 128 + 128],
                            v_sb[:, s * 2 + jt, :],
                            start=False, stop=(jt == it))
                # state update: S_nxt = elast * S_cur + k_bar^T @ v
                pd_ = pspd.tile([128, DV], F32, tag="pd")
                nc.tensor.matmul(pd_, kbar[:, s, 0:128],
                                 v_sb[:, s * 2, :], start=True, stop=False)
                nc.tensor.matmul(pd_, kbar[:, s, 128:256],
                                 v_sb[:, s * 2 + 1, :], start=False, stop=True)
                nc.vector.scalar_tensor_tensor(
                    out=s_nxt, in0=s_cur.bitcast(F32),
                    scalar=elast[:, s:s + 1], in1=pd_,
                    op0=AL.mult, op1=AL.add)
                # epilogue: rmsnorm * swish-gate; then transpose + out proj
                for it in range(2):
                    tt = s * 2 + it
                    osl = slice(it * DV, it * DV + DV)
                    scr = small.tile([128, DV], F32, tag="scr")
                    ssq = small.tile([128, 1], F32, tag="ssq")
                    nc.scalar.activation(scr, po[:, osl], ACT.Square,
                                         accum_out=ssq)
                    rstd = small.tile([128, 1], F32, tag="rstd")
                    nc.scalar.activation(rstd, ssq, ACT.Ln,
                                         bias=eps_t, scale=1.0 / DV)
                    nc.scalar.activation(rstd, rstd, ACT.Exp, scale=-0.5)
                    nc.vector.scalar_tensor_tensor(
                        out=og_sb[:, tt, :], in0=po[:, osl], scalar=rstd,
                        in1=sg_sb[:, tt, :], op0=AL.mult, op1=AL.mult)
                for it in range(2):
                    tt = s * 2 + it
                    xsl = slice(tt * 128, (tt + 1) * 128)
                    ptp = pstp.tile([128, SC], F32, tag="tp")
                    for k2 in range(2):
                        nc.tensor.matmul(
                            ptp[:, k2 * 128:k2 * 128 + 128],
                            og_sb[:, tt, k2 * 128:k2 * 128 + 128],
                            ident, start=True, stop=True)
                    nc.vector.tensor_copy(
                        ogt[:, :, xsl],
                        ptp[:, :].rearrange("p (a b) -> p a b", a=2))
                    for nb in range(2):
                        nsl = slice(nb * 512, nb * 512 + 512)
                        pout = psout.tile([128, 512], F32, tag="pout")
                        for k2 in range(2):
                            nc.tensor.matmul(
                                pout, ogt[:, k2, xsl], wo_sb[:, k2, nsl],
                                start=(k2 == 0), stop=(k2 == 1))
                        st = stage.tile([128, 512], F32, tag="st")
                        cp2 = (nc.scalar.copy if nb == 0
                               else nc.vector.tensor_copy)
                        cp2(st, pout)
                        nc.sync.dma_start(out_d[xsl, nsl], st)
    nc.finalize()
    return nc


def _get_nc():
    if "nc" not in _CACHE:
        _CACHE["nc"] = _build_program()
    return _CACHE["nc"]


def _make_in_maps(x, Wq, Wk, Wv, Wg, Wgk1, Wgk2, bgk2, gnorm_w, Wo):
    f = np.float32
    x = np.asarray(x, f)
    Wq = np.asarray(Wq, f)
    Wk = np.asarray(Wk, f)
    Wv = np.asarray(Wv, f)
    Wg = np.asarray(Wg, f)
    Wgk1 = np.asarray(Wgk1, f)
    Wgk2 = np.asarray(Wgk2, f)
    bgk2 = np.asarray(bgk2, f)
    gnorm_w = np.asarray(gnorm_w, f)
    Wo = np.asarray(Wo, f)

    scale = f(DK) ** f(-0.5)
    wz_full = Wgk1 @ Wgk2                      # [D, KD] fused low-rank gate proj
    in_maps = []
    for c in range(8):
        b, h = c // 4, c % 4
        kd = slice(h * DK, (h + 1) * DK)
        vd = slice(h * DV, (h + 1) * DV)
        in_maps.append({
            "xt": np.ascontiguousarray(x[b].T),
            "wq": np.ascontiguousarray(Wq[:, kd] * scale),
            "wk": np.ascontiguousarray(Wk[:, kd]),
            "wv": np.ascontiguousarray(Wv[:, vd]),
            "wg": np.ascontiguousarray(Wg[:, vd]),
            "wz": np.ascontiguousarray(wz_full[:, kd]),
            "bz": np.ascontiguousarray(-bgk2[kd]).reshape(DK, 1),
            "wo": np.ascontiguousarray(Wo[vd, :] * gnorm_w[:, None]),
        })
    return in_maps


def _run(in_maps, **kwargs):
    from concourse.bass_utils import run_bass_kernel_spmd
    nc = _get_nc()
    return run_bass_kernel_spmd(nc, in_maps, core_ids=list(range(8)), **kwargs)


def kernel(x, Wq, Wk, Wv, Wg, Wgk1, Wgk2, bgk2, gnorm_w, Wo):
    in_maps = _make_in_maps(x, Wq, Wk, Wv, Wg, Wgk1, Wgk2, bgk2, gnorm_w, Wo)
    res = _run(in_maps)
    out = np.zeros((B, T, D), np.float32)
    for c in range(8):
        out[c // 4] += res.results[c]["out"]
    return out


# revision 63
# speedup vs baseline: 883.2107x; 883.2107x over previous
"""FLA gated linear attention (chunked) for Trainium2, 8-core SPMD.

Sharding: 8 cores = B(2) x H(4); each core handles one (batch, head) pair:
  - head-sliced q/k/v/g projections + low-rank gate projection (fused on host
    into one [D, DK] matrix),
  - chunked gated linear attention recurrence (superchunks of 256 positions),
  - fused RMSNorm * swish gate,
  - row-parallel output projection producing a [T, D] partial; host sums the
    4 head-partials per batch.

Self-contained: hardcodes all shapes; host-side work is only sharding/layout
(slices, transpose, folding constant diagonal scales into weight slices).
"""
import sys
sys.path.insert(0, "/opt/trn_rl_repo")

import numpy as np

B, T, D = 2, 2048, 1024
H = 4
DK, DV = 128, 256
SC, NSC = 256, 8          # superchunk size / count
KT = 8                    # 128-row k-tiles over D
TT = 16                   # 128-row t-tiles over T
NORM = 16.0               # gate logit normalizer
EPS = 1e-6

_CACHE = {}


def _build_program():
    import concourse.tile as tile
    from concourse import bacc, hw_specs, mybir
    from concourse.bass import _add_dep_helper

    # Collapse the ACT piecewise-table choice to a single combined
    # Ln+Exp+Square+Copy set (positions preserved so set ids stay valid).
    # Without this, Exp picks `exp_and_others` while Ln picks `natural_log`,
    # and the scheduler inserts a table reload on nearly every Exp<->Ln
    # transition (~50 loads, ~65us of ACT time). All activations used below
    # (Exp, Ln, Square, Copy) live in natural_log_exp_and_others.
    _keep = {"natural_log_exp_and_others"}
    _orig_tables = hw_specs.get_activation_tables("gen3")
    _filtered = {n: (s if n in _keep else set()) for n, s in _orig_tables.items()}
    bacc.get_activation_tables = lambda arch: _filtered

    FR = mybir.dt.float32r
    F32 = mybir.dt.float32
    BF = mybir.dt.bfloat16
    AL = mybir.AluOpType
    ACT = mybir.ActivationFunctionType

    nc = bacc.Bacc()

    xt_d = nc.dram_tensor("xt", (D, T), FR, kind="ExternalInput")
    wq_d = nc.dram_tensor("wq", (D, DK), FR, kind="ExternalInput")
    wk_d = nc.dram_tensor("wk", (D, DK), FR, kind="ExternalInput")
    wv_d = nc.dram_tensor("wv", (D, DV), FR, kind="ExternalInput")
    wg_d = nc.dram_tensor("wg", (D, DV), FR, kind="ExternalInput")
    wz_d = nc.dram_tensor("wz", (D, DK), FR, kind="ExternalInput")
    bz_d = nc.dram_tensor("bz", (DK, 1), F32, kind="ExternalInput")
    wo_d = nc.dram_tensor("wo", (DV, D), FR, kind="ExternalInput")
    out_d = nc.dram_tensor("out", (T, D), F32, kind="ExternalOutput")

    with tile.TileContext(nc) as tc:
        with (
            tc.tile_pool(name="consts", bufs=1) as consts,
            tc.tile_pool(name="wpool", bufs=1) as wpool,
            tc.tile_pool(name="persist", bufs=1) as pers,
            tc.tile_pool(name="small", bufs=4) as small,
            tc.tile_pool(name="xt", bufs=2) as xtp,
            tc.tile_pool(name="stage", bufs=3) as stage,
            tc.tile_pool(name="ps_proj", bufs=2, space="PSUM") as psp,
            tc.tile_pool(name="ps_tp", bufs=1, space="PSUM") as pstp,
            tc.tile_pool(name="ps_pa", bufs=1, space="PSUM") as pspa,
            tc.tile_pool(name="ps_po", bufs=1, space="PSUM") as pspo,
            tc.tile_pool(name="ps_pd", bufs=1, space="PSUM") as pspd,
            tc.tile_pool(name="ps_out", bufs=2, space="PSUM") as psout,
        ):
            # ---- constants (inline DRAM -> SBUF; DMA'd on the ACT ring so
            # they don't delay the startup-critical x/weight loads) ----
            ident_d = nc.inline_tensor(np.eye(128, dtype=np.float32), name="ident_c")
            jj = np.arange(128)[:, None]
            ii = np.arange(SC)[None, :]
            m0_np = (jj <= ii).astype(np.float32)          # j-tile 0
            m1_np = (jj + 128 <= ii).astype(np.float32)    # j-tile 1
            m0_d = nc.inline_tensor(m0_np, name="m0_c")
            m1_d = nc.inline_tensor(m1_np, name="m1_c")
            zeros_d = nc.inline_tensor(np.zeros((128, 3 * DV), np.float32),
                                       name="zeros_c")
            ident = consts.tile([128, 128], FR)
            m0 = consts.tile([128, SC], F32)
            m1 = consts.tile([128, SC], F32)
            nc.scalar.dma_start(ident, ident_d[:, :].bitcast(FR))
            nc.scalar.dma_start(m0, m0_d[:, :])
            nc.scalar.dma_start(m1, m1_d[:, :])
            eps_t = consts.tile([128, 1], F32)
            nc.vector.memset(eps_t, EPS)
            neginf = consts.tile([128, SC], F32)
            nc.vector.memset(neginf, -3.0e38)
            bz_sb = consts.tile([128, 1], F32)
            nc.scalar.dma_start(bz_sb, bz_d[:, :])
            s_abc = pers.tile([128, 3 * DV], FR)    # triple-buffered state
            nc.scalar.dma_start(s_abc, zeros_d[:, :].bitcast(FR))

            xt3 = xt_d.rearrange("(k p) t -> p k t", p=128)

            # first x quarter split and interleaved with the weight DMAs so
            # the PE can start the first q-projection ~6us in
            xqs = [xtp.tile([128, KT, 512], FR, tag="xq", name=f"xq{i}")
                   for i in range(4)]
            wq_sb = wpool.tile([128, KT, DK], FR)
            wk_sb = wpool.tile([128, KT, DK], FR)
            wz_sb = wpool.tile([128, KT, DK], FR)
            wv_sb = wpool.tile([128, KT, DV], FR)
            wg_sb = wpool.tile([128, KT, DV], FR)
            wo_sb = wpool.tile([128, 2, D], FR)
            wv3 = wv_d.rearrange("(k p) n -> p k n", p=128)
            wg3 = wg_d.rearrange("(k p) n -> p k n", p=128)
            nc.sync.dma_start(xqs[0][:, 0:4, :], xt3[:, 0:4, 0:512])
            nc.sync.dma_start(wq_sb, wq_d.rearrange("(k p) n -> p k n", p=128))
            nc.sync.dma_start(xqs[0][:, 4:8, :], xt3[:, 4:8, 0:512])
            nc.sync.dma_start(wk_sb, wk_d.rearrange("(k p) n -> p k n", p=128))
            nc.sync.dma_start(wz_sb, wz_d.rearrange("(k p) n -> p k n", p=128))
            nc.sync.dma_start(wv_sb[:, 0:4, :], wv3[:, 0:4, :])
            nc.sync.dma_start(wg_sb[:, 0:4, :], wg3[:, 0:4, :])
            nc.sync.dma_start(wv_sb[:, 4:8, :], wv3[:, 4:8, :])
            nc.sync.dma_start(wg_sb[:, 4:8, :], wg3[:, 4:8, :])

            # ---- persistent activations ----
            qg = pers.tile([128, T], FR)   # q^T, then qg^T in place
            kg = pers.tile([128, T], FR)   # k^T, then kg^T in place
            sp = pers.tile([128, T], F32)  # softplus, cumsum, exp(-G) in place
            eg = pers.tile([128, T], F32)  # exp(G)
            v_sb = pers.tile([128, TT, DV], FR)
            sg_sb = pers.tile([128, TT, DV], F32)
            kbar = pers.tile([128, NSC, SC], FR)   # [t within tile, sc, 2*DK]
            og_sb = pers.tile([128, TT, DV], FR)
            ogt = pers.tile([128, 2, T], FR)       # gated output transposed
            spl = pers.tile([128, NSC], F32)
            elast = pers.tile([128, NSC], F32)

            # ====== projections + gate prep, per T-quarter of 512 ======
            for q4 in range(4):
                tsl = slice(q4 * 512, (q4 + 1) * 512)
                xq = xqs[q4]
                if q4 + 1 < 4:
                    nc.sync.dma_start(
                        xqs[q4 + 1], xt3[:, :, (q4 + 1) * 512:(q4 + 2) * 512])
                wo3 = wo_d.rearrange("(k p) n -> p k n", p=128)
                if q4 == 1:
                    nc.sync.dma_start(wo_sb[:, :, 0:512], wo3[:, :, 0:512])
                elif q4 == 2:
                    nc.sync.dma_start(wo_sb[:, :, 512:1024], wo3[:, :, 512:1024])
                # transposed projections: q^T, k^T, z^T -> softplus
                pq = psp.tile([128, 512], F32, tag="pp")
                for k in range(KT):
                    nc.tensor.matmul(pq, wq_sb[:, k, :], xq[:, k, :],
                                     start=(k == 0), stop=(k == KT - 1))
                nc.vector.tensor_copy(qg[:, tsl], pq)
                pk = psp.tile([128, 512], F32, tag="pp")
                for k in range(KT):
                    nc.tensor.matmul(pk, wk_sb[:, k, :], xq[:, k, :],
                                     start=(k == 0), stop=(k == KT - 1))
                nc.vector.tensor_copy(kg[:, tsl], pk)
                pz = psp.tile([128, 512], F32, tag="pp")
                for k in range(KT):
                    nc.tensor.matmul(pz, wz_sb[:, k, :], xq[:, k, :],
                                     start=(k == 0), stop=(k == KT - 1))
                # sp = softplus(-(z + b)) = ln(1 + exp(-(z + b)))
                nc.scalar.activation(sp[:, tsl], pz, ACT.Exp,
                                     bias=bz_sb, scale=-1.0)
                nc.scalar.activation(sp[:, tsl], sp[:, tsl], ACT.Ln, bias=1.0)
                # per-superchunk cumsum of softplus (two superchunks/quarter)
                for s in (2 * q4, 2 * q4 + 1):
                    ssl = slice(s * SC, (s + 1) * SC)
                    nc.vector.tensor_tensor_scan(
                        sp[:, ssl], sp[:, ssl], neginf, 0.0, AL.add, AL.max)
                # SP at superchunk ends, decay factors
                sp3 = sp.rearrange("p (s c) -> p s c", c=SC)
                nc.scalar.copy(
                    spl[:, 2 * q4:2 * q4 + 2].rearrange("p (s o) -> p s o", o=1),
                    sp3[:, 2 * q4:2 * q4 + 2, SC - 1:SC])
                nc.scalar.activation(elast[:, 2 * q4:2 * q4 + 2],
                                     spl[:, 2 * q4:2 * q4 + 2],
                                     ACT.Exp, scale=-1.0 / NORM)
                # qg = q^T * exp(G); kg = k^T * exp(-G) (in place)
                nc.scalar.activation(eg[:, tsl], sp[:, tsl], ACT.Exp,
                                     scale=-1.0 / NORM)
                nc.vector.tensor_mul(qg[:, tsl], qg[:, tsl].bitcast(F32),
                                     eg[:, tsl])
                nc.scalar.activation(sp[:, tsl], sp[:, tsl], ACT.Exp,
                                     scale=1.0 / NORM)
                nc.vector.tensor_mul(kg[:, tsl], kg[:, tsl].bitcast(F32),
                                     sp[:, tsl])
                # k_bar^T = kg^T * elast, transposed to [t, dk] via PE identity
                for s in (2 * q4, 2 * q4 + 1):
                    ssl = slice(s * SC, (s + 1) * SC)
                    kbt = small.tile([128, SC], FR, tag="kbt")
                    nc.vector.tensor_scalar_mul(kbt, kg[:, ssl].bitcast(F32),
                                                elast[:, s:s + 1])
                    ptp = pstp.tile([128, SC], F32, tag="tp")
                    for i in range(2):
                        isl = slice(i * 128, (i + 1) * 128)
                        nc.tensor.transpose(ptp[:, isl].bitcast(FR),
                                            kbt[:, isl], ident)
                    nc.vector.tensor_copy(kbar[:, s, :], ptp)
                # normal-layout projections: v, g (+ silu via exp/ln chain)
                for i in range(4):
                    tt = q4 * 4 + i
                    xsl = slice(i * 128, (i + 1) * 128)
                    pv = psp.tile([128, DV], F32, tag="pp")
                    for k in range(KT):
                        nc.tensor.matmul(pv, xq[:, k, xsl], wv_sb[:, k, :],
                                         start=(k == 0), stop=(k == KT - 1))
                    nc.vector.tensor_copy(v_sb[:, tt, :], pv)
                    pg = psp.tile([128, DV], F32, tag="pp")
                    for k in range(KT):
                        nc.tensor.matmul(pg, xq[:, k, xsl], wg_sb[:, k, :],
                                         start=(k == 0), stop=(k == KT - 1))
                    nc.scalar.copy(sg_sb[:, tt, :], pg)
                    # sigma(g) = exp(-ln(1 + exp(-g))) — stays in the ln/exp
                    # act table; then silu = g * sigma(g) on DVE
                    sgs = small.tile([128, DV], F32, tag="sgs")
                    nc.scalar.activation(sgs, pg, ACT.Exp, scale=-1.0)
                    nc.scalar.activation(sgs, sgs, ACT.Ln, bias=1.0)
                    nc.scalar.activation(sgs, sgs, ACT.Exp, scale=-1.0)
                    nc.gpsimd.tensor_mul(sg_sb[:, tt, :], sgs, sg_sb[:, tt, :])


            # ====== recurrence + epilogue + output projection, per SC ======
            for s in range(NSC):
                ssl = slice(s * SC, (s + 1) * SC)
                s_cur = s_abc[:, (s % 3) * DV:(s % 3) * DV + DV]
                s_nxt = s_abc[:, ((s + 1) % 3) * DV:((s + 1) % 3) * DV + DV]
                # intra-chunk scores A^T[j, i], masked to j <= i
                pa = pspa.tile([128, 512], F32, tag="pa")
                am = small.tile([128, 2, SC], FR, tag="am")
                for jt in range(2):
                    jsl = slice(s * SC + jt * 128, s * SC + jt * 128 + 128)
                    nc.tensor.matmul(pa[:, jt * SC:jt * SC + SC],
                                     kg[:, jsl], qg[:, ssl],
                                     start=True, stop=True)
                    nc.vector.tensor_mul(am[:, jt, :],
                                         pa[:, jt * SC:jt * SC + SC],
                                         (m0, m1)[jt])
                # o = qg @ S_prev + tril(A) @ v
                po = pspo.tile([128, 512], F32, tag="po")
                for it in range(2):
                    osl = slice(it * DV, it * DV + DV)
                    isl = slice(s * SC + it * 128, s * SC + it * 128 + 128)
                    nc.tensor.matmul(po[:, osl], qg[:, isl], s_cur,
                                     start=True, stop=False)
                    for jt in range(it + 1):
                        nc.tensor.matmul(
                            po[:, osl],
                            am[:, jt, it * 128:it * 128 + 128],
                            v_sb[:, s * 2 + jt, :],
                            start=False, stop=(jt == it))
                # state update: S_nxt = elast * S_cur + k_bar^T @ v
                pd_ = pspd.tile([128, DV], F32, tag="pd")
                nc.tensor.matmul(pd_, kbar[:, s, 0:128],
                                 v_sb[:, s * 2, :], start=True, stop=False)
                nc.tensor.matmul(pd_, kbar[:, s, 128:256],
                                 v_sb[:, s * 2 + 1, :], start=False, stop=True)
                nc.vector.scalar_tensor_tensor(
                    out=s_nxt, in0=s_cur.bitcast(F32),
                    scalar=elast[:, s:s + 1], in1=pd_,
                    op0=AL.mult, op1=AL.add)
                # epilogue: rmsnorm * swish-gate
                for it in range(2):
                    tt = s * 2 + it
                    osl = slice(it * DV, it * DV + DV)
                    scr = small.tile([128, DV], F32, tag="scr")
                    ssq = small.tile([128, 1], F32, tag="ssq")
                    nc.scalar.activation(scr, po[:, osl], ACT.Square,
                                         accum_out=ssq)
                    rstd = small.tile([128, 1], F32, tag="rstd")
                    nc.scalar.activation(rstd, ssq, ACT.Ln,
                                         bias=eps_t, scale=1.0 / DV)
                    nc.scalar.activation(rstd, rstd, ACT.Exp, scale=-0.5)
                    nc.vector.scalar_tensor_tensor(
                        out=og_sb[:, tt, :], in0=po[:, osl], scalar=rstd,
                        in1=sg_sb[:, tt, :], op0=AL.mult, op1=AL.mult)
                for it in range(2):
                    tt = s * 2 + it
                    xsl = slice(tt * 128, (tt + 1) * 128)
                    ptp = pstp.tile([128, SC], F32, tag="tp")
                    for k2 in range(2):
                        nc.tensor.transpose(
                            ptp[:, k2 * 128:k2 * 128 + 128].bitcast(FR),
                            og_sb[:, tt, k2 * 128:k2 * 128 + 128],
                            ident)
                    nc.vector.tensor_copy(
                        ogt[:, :, xsl],
                        ptp[:, :].rearrange("p (a b) -> p a b", a=2))
                    for nb in range(2):
                        nsl = slice(nb * 512, nb * 512 + 512)
                        pout = psout.tile([128, 512], F32, tag="pout")
                        for k2 in range(2):
                            nc.tensor.matmul(
                                pout, ogt[:, k2, xsl], wo_sb[:, k2, nsl],
                                start=(k2 == 0), stop=(k2 == 1))
                        st = stage.tile([128, 512], F32, tag="st")
                        cp2 = (nc.scalar.copy if nb == 0
                               else nc.vector.tensor_copy)
                        cp2(st, pout)
                        nc.sync.dma_start(out_d[xsl, nsl], st)
    nc.finalize()
    return nc


def _get_nc():
    if "nc" not in _CACHE:
        _CACHE["nc"] = _build_program()
    return _CACHE["nc"]


def _make_in_maps(x, Wq, Wk, Wv, Wg, Wgk1, Wgk2, bgk2, gnorm_w, Wo):
    f = np.float32
    x = np.asarray(x, f)
    Wq = np.asarray(Wq, f)
    Wk = np.asarray(Wk, f)
    Wv = np.asarray(Wv, f)
    Wg = np.asarray(Wg, f)
    Wgk1 = np.asarray(Wgk1, f)
    Wgk2 = np.asarray(Wgk2, f)
    bgk2 = np.asarray(bgk2, f)
    gnorm_w = np.asarray(gnorm_w, f)
    Wo = np.asarray(Wo, f)

    scale = f(DK) ** f(-0.5)
    wz_full = Wgk1 @ Wgk2                      # [D, KD] fused low-rank gate proj
    in_maps = []
    for c in range(8):
        b, h = c // 4, c % 4
        kd = slice(h * DK, (h + 1) * DK)
        vd = slice(h * DV, (h + 1) * DV)
        in_maps.append({
            "xt": np.ascontiguousarray(x[b].T),
            "wq": np.ascontiguousarray(Wq[:, kd] * scale),
            "wk": np.ascontiguousarray(Wk[:, kd]),
            "wv": np.ascontiguousarray(Wv[:, vd]),
            "wg": np.ascontiguousarray(Wg[:, vd]),
            "wz": np.ascontiguousarray(wz_full[:, kd]),
            "bz": np.ascontiguousarray(-bgk2[kd]).reshape(DK, 1),
            "wo": np.ascontiguousarray(Wo[vd, :] * gnorm_w[:, None]),
        })
    return in_maps


def _run(in_maps, **kwargs):
    from concourse.bass_utils import run_bass_kernel_spmd
    nc = _get_nc()
    return run_bass_kernel_spmd(nc, in_maps, core_ids=list(range(8)), **kwargs)


def _get_exec():
    """Build (once) a reusable 8-core jitted executable around the Bass NEFF.

    Mirrors bass2jax.run_bass_via_pjrt's multi-core path but without buffer
    donation, so repeat kernel() calls reuse the compiled executable instead
    of re-tracing and re-compiling.
    """
    if "exec" in _CACHE:
        return _CACHE["exec"]
    import jax
    import numpy as _np
    from jax.sharding import Mesh, PartitionSpec
    from jax.experimental.shard_map import shard_map
    import concourse.mybir as mybir
    from concourse import bass2jax
    from concourse.bass2jax import _bass_exec_p, partition_id_tensor

    nc = _get_nc()
    n_cores = 8
    bass2jax.install_neuronx_cc_hook()
    partition_name = nc.partition_id_tensor.name if nc.partition_id_tensor else None
    in_names, out_names, out_avals, zero_outs = [], [], [], []
    for alloc in nc.m.functions[0].allocations:
        if not isinstance(alloc, mybir.MemoryLocationSet):
            continue
        name = alloc.memorylocations[0].name
        if alloc.kind == "ExternalInput":
            if name != partition_name:
                in_names.append(name)
        elif alloc.kind == "ExternalOutput":
            out_names.append(name)
            shape = tuple(alloc.tensor_shape)
            dtype = mybir.dt.np(alloc.dtype)
            out_avals.append(jax.core.ShapedArray(shape, dtype))
            zero_outs.append(_np.zeros(shape, dtype))
    n_params = len(in_names)
    all_in_names = list(in_names) + out_names
    if partition_name is not None:
        all_in_names.append(partition_name)

    def _body(*args):
        operands = list(args)
        if partition_name is not None:
            operands.append(partition_id_tensor())
        outs = _bass_exec_p.bind(
            *operands,
            out_avals=tuple(out_avals),
            in_names=tuple(all_in_names),
            out_names=tuple(out_names),
            lowering_input_output_aliases=(),
            sim_require_finite=True,
            sim_require_nnan=True,
            nc=nc,
        )
        return tuple(outs)

    devices = jax.devices()[:n_cores]
    mesh = Mesh(_np.asarray(devices), ("core",))
    in_specs = (PartitionSpec("core"),) * (n_params + len(out_names))
    out_specs = (PartitionSpec("core"),) * len(out_names)
    fn = jax.jit(shard_map(_body, mesh=mesh, in_specs=in_specs,
                           out_specs=out_specs, check_rep=False),
                 keep_unused=True)
    concat_zero = [
        _np.zeros((n_cores * z.shape[0],) + z.shape[1:], z.dtype)
        for z in zero_outs
    ]
    _CACHE["exec"] = (fn, in_names, out_names, concat_zero, n_cores)
    return _CACHE["exec"]


def kernel(x, Wq, Wk, Wv, Wg, Wgk1, Wgk2, bgk2, gnorm_w, Wo):
    import jax
    in_maps = _make_in_maps(x, Wq, Wk, Wv, Wg, Wgk1, Wgk2, bgk2, gnorm_w, Wo)
    fn, in_names, out_names, concat_zero, n_cores = _get_exec()
    concat_in = [
        np.concatenate([np.asarray(in_maps[c][nm]) for c in range(n_cores)],
                       axis=0)
        for nm in in_names
    ]
    outs = fn(*concat_in, *concat_zero)
    jax.block_until_ready(outs)
    o = np.asarray(outs[out_names.index("out")]).reshape(n_cores, T, D)
    out = np.zeros((B, T, D), np.float32)
    for c in range(n_cores):
        out[c // 4] += o[c]
    return out
